# revision 6
# baseline (speedup 1.0000x reference)
"""CostVolume kernel for Trainium2 (8 NeuronCores, Bass/Tile).

Problem: x, y [B=4, C=320, H=128, W=240] fp32.
out[0, 0, d*B + b, h, w] = sum_c x[b,c,h,w] * y[b,c,h,w-d]  for d in [0, 49),
zero where w < d.

Strategy:
- Shard (b, h-half) across 8 cores; each core gets x/y [320, 64, 240].
- Per h row, the needed values form the 49-diagonal band of the Gram
  matrix Z[w1, w2] = sum_c x[c,w1] y[c,w2].  TensorE computes band
  rectangles: two w1 chunks (128 / 112 wide), each with its w2 window
  (w1lo-48 .. w1hi), with y zero-padded by 48 columns on the left so the
  window is always in range and the zero entries reproduce the
  reference's left-padding.
- bf16 matmuls (cast during SWDGE DMA), fp32 PSUM accumulate over the 3
  c-chunks (128+128+64).
- PSUM [128, 336] -> SBUF staging -> DRAM raw [128, 64, 336] per core.
- Host-side unshard: pure as_strided diagonal reindex (no arithmetic)
  into the [1, 1, 196, 128, 240] output.
"""

import numpy as np

B, C, H, W = 4, 320, 128, 240
D = 49  # maxdisp 48 + 1
HSH = 64  # h rows per core (half of H)
NCORES = 8
PAD = 48  # zero columns prepended to y
# w1 chunks: [0, 128) and [128, 240)
Q0_M, Q1_M = 128, 112
Q0_N, Q1_N = 176, 160  # w2 windows: [-48, 128) and [80, 240)
RAW_COLS = Q0_N + Q1_N  # 336
HB = 8  # h rows per DMA block


def _build_program(repeat: int = 1):
    import concourse.tile as tile
    from concourse import bacc, mybir

    nc = bacc.Bacc(None)
    x_d = nc.declare_dram_parameter("x", [C, HSH, W], mybir.dt.float32, isOutput=False)
    y_d = nc.declare_dram_parameter("y", [C, HSH, W], mybir.dt.float32, isOutput=False)
    raw_d = nc.declare_dram_parameter(
        "raw", [Q0_M, HSH, RAW_COLS], mybir.dt.float32, isOutput=True
    )

    KCH = [(0, 128), (128, 128), (256, 64)]  # c chunk starts / sizes

    with tile.TileContext(nc) as tc:
        with (
            tc.tile_pool(name="xin", bufs=2) as xpool,
            tc.tile_pool(name="yin", bufs=2) as ypool,
            tc.tile_pool(name="stg", bufs=2) as spool,
            tc.tile_pool(name="ps", bufs=2, space="PSUM") as ppool,
        ):
            for blk in range(repeat * (HSH // HB)):
                h0 = (blk % (HSH // HB)) * HB
                x_sb = xpool.tile([128, 3, HB, W], mybir.dt.bfloat16)
                y_sb = ypool.tile([128, 3, HB, PAD + W], mybir.dt.bfloat16)
                # zero pad region (left 48 cols of each (k, h) row of y)
                nc.vector.memset(y_sb[:, :, :, 0:PAD], 0.0)
                for k, (c0, csz) in enumerate(KCH):
                    nc.gpsimd.dma_start(
                        out=x_sb[0:csz, k, :, :],
                        in_=x_d[c0 : c0 + csz, h0 : h0 + HB, :],
                    )
                    nc.gpsimd.dma_start(
                        out=y_sb[0:csz, k, :, PAD : PAD + W],
                        in_=y_d[c0 : c0 + csz, h0 : h0 + HB, :],
                    )
                stage = spool.tile([128, HB, RAW_COLS], mybir.dt.float32)
                for hh in range(HB):
                    psum0 = ppool.tile([128, Q0_N], mybir.dt.float32, tag="ps0")
                    psum1 = ppool.tile([Q1_M, Q1_N], mybir.dt.float32, tag="ps1")
                    for k, (c0, csz) in enumerate(KCH):
                        st, sp = (k == 0), (k == 2)
                        # q0: w1 in [0,128), w2 in [-48,128) -> ypad cols [0,176)
                        nc.tensor.matmul(
                            psum0[:, :],
                            lhsT=x_sb[0:csz, k, hh, 0:Q0_M],
                            rhs=y_sb[0:csz, k, hh, 0:Q0_N],
                            start=st,
                            stop=sp,
                        )
                        # q1: w1 in [128,240), w2 in [80,240) -> ypad cols [128,288)
                        nc.tensor.matmul(
                            psum1[:, :],
                            lhsT=x_sb[0:csz, k, hh, Q0_M:W],
                            rhs=y_sb[0:csz, k, hh, Q0_M : Q0_M + Q1_N],
                            start=st,
                            stop=sp,
                        )
                    nc.vector.tensor_copy(out=stage[:, hh, 0:Q0_N], in_=psum0[:, :])
                    nc.vector.tensor_copy(
                        out=stage[0:Q1_M, hh, Q0_N:RAW_COLS], in_=psum1[:, :]
                    )
                nc.sync.dma_start(
                    out=raw_d[:, h0 : h0 + HB, :],
                    in_=stage[:, :, :],
                )
    nc.finalize()
    return nc


_NC_CACHE = None


def _get_program():
    global _NC_CACHE
    if _NC_CACHE is None:
        _NC_CACHE = _build_program()
    return _NC_CACHE


def _extract(raw: np.ndarray) -> np.ndarray:
    """raw [128, HSH, 336] -> out [D, HSH, W] via pure stride tricks."""
    assert raw.shape == (Q0_M, HSH, RAW_COLS) and raw.dtype == np.float32
    s_r, s_h, s_c = raw.strides
    # region A: w' in [0, 128): out[d, h, w'] = raw[w', h, (w' - d) + 48]
    a = np.lib.stride_tricks.as_strided(
        raw[:, :, PAD:],  # base offset col=48 (d=0)
        shape=(D, HSH, Q0_M),
        strides=(-s_c, s_h, s_r + s_c),
    )
    # region B: w' in [128, 240): out[d, h, w'] = raw[w'-128, h, 176 + (w'-d) - 80]
    # at w'=128: col = 176 + 48 - d
    b = np.lib.stride_tricks.as_strided(
        raw[:, :, Q0_N + PAD :],
        shape=(D, HSH, Q1_M),
        strides=(-s_c, s_h, s_r + s_c),
    )
    out = np.empty((D, HSH, W), np.float32)
    out[:, :, :Q0_M] = a
    out[:, :, Q0_M:] = b
    return out


def kernel(x: np.ndarray, y: np.ndarray) -> np.ndarray:
    from concourse.bass_utils import run_bass_kernel_spmd

    x = np.asarray(x, dtype=np.float32)
    y = np.asarray(y, dtype=np.float32)
    assert x.shape == (B, C, H, W) and y.shape == (B, C, H, W)

    in_maps = []
    for core in range(NCORES):
        b, half = core // 2, core % 2
        h0 = half * HSH
        in_maps.append(
            {
                "x": np.ascontiguousarray(x[b, :, h0 : h0 + HSH, :]),
                "y": np.ascontiguousarray(y[b, :, h0 : h0 + HSH, :]),
            }
        )

    nc = _get_program()
    res = run_bass_kernel_spmd(nc, in_maps, core_ids=list(range(NCORES)))

    out = np.empty((D, B, H, W), np.float32)
    for core in range(NCORES):
        b, half = core // 2, core % 2
        h0 = half * HSH
        raw = np.asarray(res.results[core]["raw"], dtype=np.float32)
        out[:, b, h0 : h0 + HSH, :] = _extract(raw)
    return out.reshape(1, 1, D * B, H, W)


# revision 7
# speedup vs baseline: 1.3280x; 1.3280x over previous
"""CostVolume kernel for Trainium2 (8 NeuronCores, Bass/Tile).

Problem: x, y [B=4, C=320, H=128, W=240] fp32.
out[0, 0, d*B + b, h, w] = sum_c x[b,c,h,w] * y[b,c,h,w-d]  for d in [0, 49),
zero where w < d.

This environment charges a large fixed cost per *instruction* (~25-70us,
measured), so the design minimizes instruction count rather than classic
roofline terms:

- Shard (b, w-half) across 8 cores; each core: x [320, 128, 120] and a
  pre-padded y window [320, 128, 168] (host supplies zeros for w < 0, so
  the zero-padding of the reference falls out naturally).
- SBUF layout [h=128 partitions, c, w]; disparity shift d is a uniform
  byte offset on the free (w) axis, so one big DVE tensor_mul computes
  x * shift_d(y) for TWO disparities at once (4D AP with a -1 element
  stride over d), and one DVE tensor_reduce sums over c (via a
  transposed AP). C=320 is processed in 2 chunks of 160 (SBUF capacity),
  chunk 1 accumulated with a tensor_add.
- ~131 instructions total per core; no PSUM, no matmuls.
"""

import numpy as np

B, C, H, W = 4, 320, 128, 240
D = 49  # maxdisp 48 + 1
NCORES = 8
WSH = W // 2  # 120 output columns per core
YW = WSH + 48  # 168 y columns incl. 48-halo (zeros for core 0's left edge)
CC = 160  # c-chunk size
DP = 2  # disparities per DVE instruction


def _build_program(repeat: int = 1):
    import concourse.bass as bass
    import concourse.tile as tile
    from concourse import bacc, mybir

    nc = bacc.Bacc(None)
    x_d = nc.declare_dram_parameter("x", [C, H, WSH], mybir.dt.float32, isOutput=False)
    y_d = nc.declare_dram_parameter("y", [C, H, YW], mybir.dt.float32, isOutput=False)
    # h-major raw output; host transposes to [d, h, w]
    out_d = nc.declare_dram_parameter("o", [H, D, WSH], mybir.dt.float32, isOutput=True)

    with tile.TileContext(nc) as tc:
        with (
            tc.tile_pool(name="xin", bufs=1) as xpool,
            tc.tile_pool(name="yin", bufs=1) as ypool,
            tc.tile_pool(name="pr", bufs=1) as prpool,
            tc.tile_pool(name="ot", bufs=1) as opool,
            tc.tile_pool(name="tm", bufs=2) as tpool,
        ):
            for rep in range(repeat):
                out_sb = opool.tile([H, D, WSH], mybir.dt.float32, tag="out")
                for cc in range(C // CC):
                    c0 = cc * CC
                    x_sb = xpool.tile([H, CC, WSH], mybir.dt.bfloat16, tag="x")
                    y_sb = ypool.tile([H, CC, YW], mybir.dt.bfloat16, tag="y")
                    # cast-DMA fp32->bf16; split c to stay under the
                    # SWDGE descriptor budget (<=16384 per dma_start)
                    hc = CC // 2
                    for j in range(2):
                        nc.gpsimd.dma_start(
                            out=x_sb[:, j * hc : (j + 1) * hc, :],
                            in_=x_d[c0 + j * hc : c0 + (j + 1) * hc, :, :].transpose(
                                [1, 0, 2]
                            ),
                        )
                        nc.gpsimd.dma_start(
                            out=y_sb[:, j * hc : (j + 1) * hc, :],
                            in_=y_d[c0 + j * hc : c0 + (j + 1) * hc, :, :].transpose(
                                [1, 0, 2]
                            ),
                        )
                    for dp in range((D + DP - 1) // DP):
                        d0 = dp * DP
                        dsz = min(DP, D - d0)
                        prod = prpool.tile([H, DP, CC, WSH], mybir.dt.bfloat16, tag="p")
                        # in1: y[h, c, (48 - d) + w] for d = d0..d0+dsz-1
                        base = y_sb[:, :, 48 - d0 : 48 - d0 + WSH]
                        in1 = bass.AP(
                            tensor=base.tensor,
                            offset=base.offset,
                            ap=[base.ap[0], [-1, dsz], base.ap[1], base.ap[2]],
                        )
                        xa = x_sb[:, :, :]
                        in0 = bass.AP(
                            tensor=xa.tensor,
                            offset=xa.offset,
                            ap=[xa.ap[0], [0, dsz], xa.ap[1], xa.ap[2]],
                        )
                        nc.vector.tensor_mul(prod[:, 0:dsz, :, :], in0, in1)
                        red_in = prod[:, 0:dsz, :, :].transpose([0, 1, 3, 2])
                        if cc == 0:
                            nc.vector.tensor_reduce(
                                out=out_sb[:, d0 : d0 + dsz, :],
                                in_=red_in,
                                axis=mybir.AxisListType.X,
                                op=mybir.AluOpType.add,
                            )
                        else:
                            tmp = tpool.tile([H, DP, WSH], mybir.dt.float32, tag="t")
                            nc.vector.tensor_reduce(
                                out=tmp[:, 0:dsz, :],
                                in_=red_in,
                                axis=mybir.AxisListType.X,
                                op=mybir.AluOpType.add,
                            )
                            nc.vector.tensor_add(
                                out=out_sb[:, d0 : d0 + dsz, :],
                                in0=out_sb[:, d0 : d0 + dsz, :],
                                in1=tmp[:, 0:dsz, :],
                            )
                nc.sync.dma_start(out=out_d[:, :, :], in_=out_sb[:, :, :])
    nc.finalize()
    return nc


_NC_CACHE = None


def _get_program():
    global _NC_CACHE
    if _NC_CACHE is None:
        _NC_CACHE = _build_program()
    return _NC_CACHE


def _shard_inputs(x: np.ndarray, y: np.ndarray):
    in_maps = []
    for core in range(NCORES):
        b, wh = core // 2, core % 2
        xs = np.ascontiguousarray(x[b, :, :, wh * WSH : (wh + 1) * WSH])
        lo = wh * WSH - 48
        if lo < 0:
            ys = np.concatenate(
                [np.zeros((C, H, -lo), np.float32), y[b, :, :, : wh * WSH + WSH]],
                axis=2,
            )
        else:
            ys = y[b, :, :, lo : wh * WSH + WSH]
        in_maps.append({"x": xs, "y": np.ascontiguousarray(ys)})
    return in_maps


def kernel(x: np.ndarray, y: np.ndarray) -> np.ndarray:
    from concourse.bass_utils import run_bass_kernel_spmd

    x = np.asarray(x, dtype=np.float32)
    y = np.asarray(y, dtype=np.float32)
    assert x.shape == (B, C, H, W) and y.shape == (B, C, H, W)

    in_maps = _shard_inputs(x, y)
    nc = _get_program()
    res = run_bass_kernel_spmd(nc, in_maps, core_ids=list(range(NCORES)))

    out = np.empty((D, B, H, W), np.float32)
    for core in range(NCORES):
        b, wh = core // 2, core % 2
        raw = np.asarray(res.results[core]["o"], dtype=np.float32)  # [H, D, WSH]
        out[:, b, :, wh * WSH : (wh + 1) * WSH] = raw.transpose(1, 0, 2)
    return out.reshape(1, 1, D * B, H, W)
